# revision 11
# baseline (speedup 1.0000x reference)
"""e3nn-style 5x5x5 SAME conv3d ([2,32,32,32,32] -> [2,32,32,32,288]) on 8 trn2 cores.

Sharding: batch(2) x X-chunks(4) -> 8 cores; each core gets a zero-padded input
slab with 2-voxel halos and computes its [8,32,32,288] output slice.

Per-core algorithm (implicit GEMM, fp8 DoubleRow + fp16 center):
  - The reference kernel is dominated by the center tap (w_lin/fan, ~97% of
    output variance); the 124 off-center taps are attenuated by 1/nvox=1/125
    and carry only ~3% of the variance. So the off-center conv runs entirely
    in fp8e4m3 (measured rel err 6.9e-3 < 2e-2 gate) while the center tap is
    a separate fp16 matmul.
  - Off-center conv: taps packed 4-per-K by replicating the input slab 4x
    across partition groups with shifts (A: z-shifts, B: y-shifts at dz=4,
    C: x-shifts at dy=dz=4), giving 32 K=128 packs; DoubleRow then pairs
    packs two-at-a-time (K_eff=256, 0.5 cycles/row): 16 DR matmuls of
    [K=256, M=128 voxels] x [K=256, N=288] per 128-voxel PSUM block.
    Pair members are addressed inside one fp8 mega-slab via hand-built
    3D APs [K=128, (2, pair_stride), (128, 1)] (all strides %16==0).
  - Center: one [K=32, M=128] x [K=32, N=32] fp16 matmul per block
    accumulating w_lin^T x into the l=0 output channels.
  - All weights are pre-scaled by 2^14 (keeps fp8 out of subnormals); the
    PSUM->SBUF copy descales by 2^-14 for free (ACT scalar.mul) and emits
    fp16, halving the output DMA. Weights are synthesized on the host
    (tiny einsum) and DMA'd interleaved with the slabs in consumption
    order. Cost model: 64 blocks x (16x60ns DR + 15ns center) ~= 64us
    TensorE + ~8us head, vs 246us for the all-fp16 direct conv.
"""

import numpy as np
import ml_dtypes

try:
    import concourse.bass as bass  # noqa: F401
except ImportError:
    import sys

    sys.path.insert(0, "/opt/trn_rl_repo")

import concourse.mybir as mybir
import concourse.tile as tile
from concourse import bacc
from concourse.bass_utils import run_bass_kernel_spmd

F32 = mybir.dt.float32
F16 = mybir.dt.float16
FP8 = mybir.dt.float8e4

GRID = 32
CIN = 32
COUT = 288
NRB = 8
XPER = 8  # output x-planes per core
XS = 12  # slab x extent (XPER + 2*2 halo)
MLS = (1, 3, 5)
LOFF_OUT = (0, 32, 128)  # l-block offsets in the 288 output channels
FAN = float(np.sqrt(32.0))
NVOX = 125.0
SCALE = 16384.0  # 2^14: max conv |w|*SCALE ~ 173 < 240 (e4m3 max)

# mega-slab is x-major: per x-plane [A_x (1152) ++ B_x (1024) ++ C_x (1024)],
# C present only for x<9 (C packs read xo+xb <= 8)
def _xoff(x):
    return 3200 * x if x <= 8 else 28800 + 2176 * (x - 9)


MEGA_N = 28800 + 3 * 2176  # 35328 fp8 elems per partition

# DoubleRow pair table: 16 pairs of pack codes; A(dx,dy)=5dx+dy, B(dx)=25+dx,
# C(xb)=30+xb. Order is also the rall8 layout order.
PAIRS = (
    [(0 * 5 + dy, 1 * 5 + dy) for dy in range(5)]
    + [(2 * 5 + dy, 3 * 5 + dy) for dy in range(5)]
    + [(20, 21), (22, 23), (24, 29), (25, 26), (27, 28), (30, 31)]
)


def _pack_off(b, xo, yb):
    """Mega-slab element offset of pack b's stationary window at (xo, yb)."""
    if b < 25:
        dx, dy = divmod(b, 5)
        return _xoff(xo + dx) + 32 * (4 * yb + dy)
    if b < 30:
        return _xoff(xo + (b - 25)) + 1152 + 128 * yb
    return _xoff(xo + (b - 30)) + 2176 + 128 * yb


def _tap_of(b, r):
    """Flat tap index covered by weight-base b at partition group r, or None."""
    if b < 25:  # slab A: z-shifted replicas, taps (dx, dy, dz=r)
        dx, dy = divmod(b, 5)
        if (dx, dy, r) == (2, 2, 2):
            return None  # center tap: separate fp16 matmul
        return (dx * 5 + dy) * 5 + r
    if b < 30:  # slab B: y-shifted replicas at dz=4, taps (dx, dy=r, 4)
        dx = b - 25
        return (dx * 5 + r) * 5 + 4
    # slab C: x-shifted replicas at dy=4, dz=4, taps (dx=xb+r, 4, 4)
    xb = b - 30
    if xb == 1 and r < 3:
        return None  # duplicate of the xb=0 bar
    dx = xb + r
    return (dx * 5 + 4) * 5 + 4


def _host_rall(w0, w1, w2, w_lin):
    """R_all[(r,u), b*288 + outch]: the full tap-packed conv weights (f32).

    R_all[32r+u, 288b + LOFF_OUT[l] + ml*v + m] =
        emb[tap(b,r), k]/(nvox*fan) * w_l[k,u,v] * Y_l[tap(b,r), m]
    """
    c = np.arange(-2.0, 3.0)
    lat = np.stack(np.meshgrid(c, c, c, indexing="ij"), axis=-1).reshape(125, 3)
    rad = np.linalg.norm(lat, axis=-1)
    u = lat / np.where(rad == 0.0, 1.0, rad)[:, None]
    ux, uy, uz = u[:, 0], u[:, 1], u[:, 2]

    y0 = np.ones((125, 1))
    y1 = np.sqrt(3.0) * np.stack([uy, uz, ux], axis=-1)
    y2 = np.stack(
        [
            np.sqrt(15.0) * ux * uy,
            np.sqrt(15.0) * uy * uz,
            (np.sqrt(5.0) / 2.0) * (2.0 * uz**2 - ux**2 - uy**2),
            np.sqrt(15.0) * ux * uz,
            (np.sqrt(15.0) / 2.0) * (ux**2 - uy**2),
        ],
        axis=-1,
    )
    ys = (y0, y1, y2)

    # e3nn soft_one_hot_linspace, basis='smooth_finite'
    values = np.linspace(0.0, 2.5, NRB + 2)
    step = values[1] - values[0]
    values = values[1:-1]
    d = (rad[:, None] - values[None, :]) / step

    def sus(x):
        return np.where(x > 0.0, np.exp(-1.0 / np.where(x > 0.0, x, 1.0)), 0.0)

    emb = 1.14136 * np.exp(2.0) * sus(d + 1.0) * sus(1.0 - d)  # [125, 8]
    emb = emb / (NVOX * FAN)

    wl = [
        np.einsum("tk,kuv->tuv", emb, np.asarray(w, np.float64))
        for w in (w0, w1, w2)
    ]
    rall = np.zeros((128, 32 * COUT), np.float64)
    for b in range(32):
        for r in range(4):
            t = _tap_of(b, r)
            if t is None:
                continue
            for l in range(3):
                ml = MLS[l]
                blk = wl[l][t][:, :, None] * ys[l][t][None, None, :]
                lo = b * COUT + LOFF_OUT[l]
                rall[32 * r : 32 * (r + 1), lo : lo + 32 * ml] = blk.reshape(32, -1)
    return rall.astype(np.float32)


def _build_nc(repeat=1):
    nc = bacc.Bacc("TRN2", target_bir_lowering=False, debug=False)

    mega_d = nc.dram_tensor("mega", [128, MEGA_N], FP8, kind="ExternalInput")
    rall8_d = nc.dram_tensor("rall8", [128, 16 * 2 * COUT], FP8, kind="ExternalInput")
    a16_d = nc.dram_tensor("a16", [32, 8 * 1152], F16, kind="ExternalInput")
    wlin_d = nc.dram_tensor("wlin", [32, 32], F16, kind="ExternalInput")
    out_d = nc.dram_tensor("out", [XPER * 32 * 32, COUT], F16, kind="ExternalOutput")

    DR = mybir.MatmulPerfMode.DoubleRow

    with tile.TileContext(nc) as tc:
        with (
            tc.tile_pool(name="wts", bufs=1) as wts_pool,
            tc.tile_pool(name="slab", bufs=1) as slab_pool,
            tc.tile_pool(name="stage", bufs=4) as stage_pool,
            tc.tile_pool(name="ps", bufs=8, space="PSUM") as ps_pool,
        ):
            mega = slab_pool.tile([128, MEGA_N], FP8, tag="mega", name="mega")
            rall8 = wts_pool.tile([128, 16 * 2 * COUT], FP8, tag="r8", name="rall8")
            a16 = wts_pool.tile([32, 8 * 1152], F16, tag="a16", name="a16")
            wlin = wts_pool.tile([32, 32], F16, tag="wlin", name="wlin")

            def dma_mega(lo, hi):
                nc.sync.dma_start(mega[:, lo:hi], mega_d[:, lo:hi])

            def dma_r8(p0, p1):
                nc.sync.dma_start(
                    rall8[:, p0 * 576 : p1 * 576], rall8_d[:, p0 * 576 : p1 * 576]
                )

            # consumption-order interleave: per-x mega chunks with rall8
            # quarters; wlin/a16 before the last quarter (block-0 center)
            dma_mega(_xoff(0), _xoff(1))
            dma_mega(_xoff(1), _xoff(2))
            dma_r8(0, 4)
            dma_mega(_xoff(2), _xoff(3))
            dma_mega(_xoff(3), _xoff(4))
            dma_r8(4, 8)
            dma_mega(_xoff(4), _xoff(5))
            dma_mega(_xoff(5), _xoff(6))
            dma_r8(8, 12)
            nc.sync.dma_start(wlin[:], wlin_d[:])
            nc.sync.dma_start(a16[:], a16_d[:])
            dma_r8(12, 16)
            for x in range(6, XS):
                dma_mega(_xoff(x), _xoff(x + 1) if x + 1 < XS else MEGA_N)

            r8v = rall8.rearrange("p (q n) -> p q n", n=2 * COUT)  # [128,16,576]

            def pair_ap(off0, off1):
                ap = mega[:, off0 : off0 + 128].unsqueeze(1)
                s = off1 - off0
                assert s > 0 and s % 16 == 0, s
                ap.ap[1] = [s, 2]
                return ap

            for _rep in range(repeat):
              for xo in range(XPER):
                for yb2 in range(4):  # 2 blocks share one staging tile + DMA
                    stg = stage_pool.tile([128, 2 * COUT], F16, tag="stg", name="stg")
                    for j in range(2):
                        yb = 2 * yb2 + j
                        ps_blk = ps_pool.tile([128, COUT], F32, tag="ps", name="ps_blk")
                        for p, (b0, b1) in enumerate(PAIRS):
                            nc.tensor.matmul(
                                ps_blk[:, :],
                                pair_ap(_pack_off(b0, xo, yb), _pack_off(b1, xo, yb)),
                                r8v[:, p, :].rearrange("p (q n) -> p q n", n=COUT),
                                start=(p == 0),
                                stop=False,
                                perf_mode=DR,
                            )
                        # center tap: w_lin^T x into the l=0 channels (fp16)
                        o = xo * 1152 + 32 * (4 * yb + 2)
                        nc.tensor.matmul(
                            ps_blk[:, 0:32],
                            a16[:, o : o + 128],
                            wlin[:, :],
                            start=False,
                            stop=True,
                        )
                        nc.scalar.mul(
                            stg[:, j * COUT : (j + 1) * COUT], ps_blk[:], 1.0 / SCALE
                        )
                    row = xo * 1024 + yb2 * 256
                    nc.sync.dma_start(
                        out_d[row : row + 256, :].rearrange("(j p) n -> p j n", j=2),
                        stg.rearrange("p (j n) -> p j n", n=COUT),
                    )

    nc.compile()
    return nc


def _shard_inputs(x, w0, w1, w2, w_lin):
    rall = _host_rall(w0, w1, w2, w_lin)
    rall8 = np.zeros((128, 16 * 2 * COUT), ml_dtypes.float8_e4m3)
    for p, pair in enumerate(PAIRS):
        for q, b in enumerate(pair):
            rall8[:, (p * 2 + q) * COUT : (p * 2 + q + 1) * COUT] = (
                rall[:, b * COUT : (b + 1) * COUT] * SCALE
            ).astype(ml_dtypes.float8_e4m3)
    wlin16 = (np.asarray(w_lin, np.float64) * (SCALE / FAN)).astype(np.float16)

    in_maps = []
    for core in range(8):
        bb, xi = divmod(core, 4)
        x0 = xi * XPER
        pp = np.zeros((CIN, XS, 36, 36), np.float32)
        glo, ghi = x0 - 2, x0 + XPER + 2
        slo, shi = max(glo, 0), min(ghi, GRID)
        pp[:, slo - glo : shi - glo, 2:34, 2:34] = x[bb, slo:shi].transpose(3, 0, 1, 2)
        p4a = np.stack([pp[:, :, :, r : r + 32] for r in range(4)], axis=0)
        p4b = np.stack([pp[:, :, r : r + 32, 4:36] for r in range(4)], axis=0)
        p4c = np.stack([pp[:, r : r + 9, 4:36, 4:36] for r in range(4)], axis=0)
        fa = np.ascontiguousarray(p4a).reshape(128, XS, 1152)
        fb = np.ascontiguousarray(p4b).reshape(128, XS, 1024)
        fc = np.ascontiguousarray(p4c).reshape(128, 9, 1024)
        # x-major mega: per x-plane [A_x ++ B_x ++ C_x(<9)]
        parts = []
        for xx in range(XS):
            parts.append(fa[:, xx])
            parts.append(fb[:, xx])
            if xx < 9:
                parts.append(fc[:, xx])
        mega = np.concatenate(parts, axis=1).astype(ml_dtypes.float8_e4m3)
        assert mega.shape[1] == MEGA_N
        a16 = (
            np.ascontiguousarray(p4a[2][:, 2:10]).reshape(32, -1).astype(np.float16)
        )
        in_maps.append(
            {
                "mega": mega,
                "rall8": rall8,
                "a16": a16,
                "wlin": wlin16,
            }
        )
    return in_maps


_NC = None


def _run(x, w0, w1, w2, w_lin, **spmd_kwargs):
    global _NC
    if _NC is None:
        _NC = _build_nc()
    in_maps = _shard_inputs(
        np.asarray(x, np.float32),
        np.asarray(w0, np.float32),
        np.asarray(w1, np.float32),
        np.asarray(w2, np.float32),
        np.asarray(w_lin, np.float32),
    )
    res = run_bass_kernel_spmd(_NC, in_maps, core_ids=list(range(8)), **spmd_kwargs)
    out = np.empty((2, GRID, GRID, GRID, COUT), np.float32)
    for core in range(8):
        bb, xi = divmod(core, 4)
        out[bb, xi * XPER : (xi + 1) * XPER] = (
            res.results[core]["out"].astype(np.float32).reshape(XPER, GRID, GRID, COUT)
        )
    return out, res


def kernel(x, w0, w1, w2, w_lin):
    out, _ = _run(x, w0, w1, w2, w_lin)
    return out
